# revision 5
# baseline (speedup 1.0000x reference)
"""GNN message-passing (segment-softmax attention aggregation) on 8 TRN2 cores.

Strategy v3 (edge-expanded X streaming — no dma_gather):
- Nodes sorted by degree -> canonical positions; group g = pos//128 owned by
  core g%8.  Each core owns NLOC nodes in NK groups of 128.
- Host builds, per core, an edge-expanded table Xe[D, S] (bf16): column
  (colbase[k] + j)*128 + p  holds X[dst of edge j of node p in group k],
  padded to the common (max-over-cores) per-group degree d_sched[k].
- Device: Q for own nodes via matmul (resident bf16 in SBUF).  Main loop
  streams Xe in 4KB-per-partition slabs; per (group k, edge j) a 128x128
  LDW+MM pair computes K|V for all 128 nodes of the group in node-major PSUM
  layout.  ACT casts K|V to bf16 SBUF (V stored h-major); DVE computes q.k
  scores (bf16 2x mult + 1x reduce), ACT exp with additive -1e30 pad mask,
  DVE e*V (bf16 2x); per-group reduce over j gives num, batched tail ops per
  8 groups give out = num/max(den,eps).
- Host reassembles the full [N, H] output from the 8 per-core outputs.

Softmax max-subtraction is skipped: scores are small here, exp is safe in
fp32 and softmax is shift-invariant, so results match to fp32 rounding.
"""

import math
import sys

import ml_dtypes
import numpy as np

for _p in ("/opt/trn_rl_repo", "/root/.axon_site/_ro/trn_rl_repo"):
    if _p not in sys.path:
        sys.path.append(_p)

P = 128
NC = 8
JC = 16           # psum chunk: j-slots per batch
NEG = -1.0e30


def _cfg_from_shapes(N, D, H):
    NPOS = ((N + 1023) // 1024) * 1024
    NG = NPOS // P
    NK = NG // NC
    NLOC = NK * P
    return dict(N=N, D=D, H=H, NPOS=NPOS, NG=NG, NK=NK, NLOC=NLOC)


def _prep(cfg, X, Wq, Wk, Wv, edge_index):
    N, D, H = cfg["N"], cfg["D"], cfg["H"]
    NPOS, NK, NLOC = cfg["NPOS"], cfg["NK"], cfg["NLOC"]
    NDUM = NPOS - N
    BF = ml_dtypes.bfloat16

    src = np.asarray(edge_index[0], dtype=np.int64)
    dst = np.asarray(edge_index[1], dtype=np.int64)
    E = src.shape[0]

    deg = np.bincount(src, minlength=N)
    order = np.argsort(deg, kind="stable")          # real nodes, degree asc
    pos_of = np.empty(N, np.int64)
    pos_of[order] = NDUM + np.arange(N)

    spos = pos_of[src]
    ecore = (spos // P) % NC
    sloc = (spos // (P * NC)) * P + spos % P        # canonical-local row

    # per (core, loc) degree -> common per-group schedule
    key = ecore * NLOC + sloc
    deg_cl = np.bincount(key, minlength=NC * NLOC).reshape(NC, NLOC)
    d_sched = deg_cl.reshape(NC, NK, P).max(axis=(0, 2))
    d_sched = np.maximum(d_sched, 1)                # [NK]
    colbase = np.concatenate(([0], np.cumsum(d_sched)))[:-1]
    TOTCOL = int(d_sched.sum())
    DMAX = int(d_sched.max())

    # per-edge slot assignment (j = rank within (core, loc))
    eorder = np.argsort(key, kind="stable")
    counts = deg_cl.ravel()
    starts = np.concatenate(([0], np.cumsum(counts)))[:-1]
    j_of = np.arange(E) - starts[key[eorder]]
    se_loc = sloc[eorder]
    se_core = ecore[eorder]
    se_dst = dst[eorder]
    se_k = se_loc // P
    se_p = se_loc % P
    se_col = colbase[se_k] + j_of                   # column in [0, TOTCOL)
    se_slot = se_col * P + se_p                     # flat slot id per core

    Xet = np.ascontiguousarray(np.asarray(X, np.float32).T.astype(BF))  # [D, N]

    kk = np.arange(NLOC)
    in_maps = []
    for c in range(NC):
        m_e = se_core == c
        slots = se_slot[m_e]
        dst_slot = np.zeros(TOTCOL * P, np.int64)
        dst_slot[slots] = se_dst[m_e]
        valid = np.zeros(TOTCOL * P, bool)
        valid[slots] = True
        xe = np.ascontiguousarray(Xet[:, dst_slot])             # [D, S]
        mask = np.where(valid.reshape(TOTCOL, P).T, 0.0,
                        np.float32(NEG)).astype(np.float32)     # [128, TOTCOL]

        gpos = ((kk // P) * NC + c) * P + kk % P                # canonical pos
        node_of_loc = np.zeros(NLOC, np.int64)
        real = gpos >= NDUM
        node_of_loc[real] = order[gpos[real] - NDUM]
        xtq = np.ascontiguousarray(Xet[:, node_of_loc])         # [D, NLOC]

        in_maps.append({"xe": xe, "xtq": xtq, "mask": mask})

    w = np.concatenate(
        [np.asarray(Wk, np.float32), np.asarray(Wv, np.float32),
         np.asarray(Wq, np.float32)], axis=1).astype(BF)        # [D, 3H]
    for m in in_maps:
        m["w"] = w

    meta = dict(d_sched=d_sched.tolist(), colbase=colbase.tolist(),
                TOTCOL=TOTCOL, DMAX=DMAX)
    post = dict(order=order, NDUM=NDUM)
    return meta, in_maps, post


def _build_program(cfg, meta):
    import concourse.bass as bass
    import concourse.tile as tile
    from concourse import bacc, mybir
    from contextlib import ExitStack

    f32 = mybir.dt.float32
    bf16 = mybir.dt.bfloat16
    AF = mybir.ActivationFunctionType
    OP = mybir.AluOpType
    AX = mybir.AxisListType

    D, H = cfg["D"], cfg["H"]
    NK, NLOC = cfg["NK"], cfg["NLOC"]
    H2 = 2 * H
    DC = D // P
    d_sched = meta["d_sched"]
    colbase = meta["colbase"]
    TOTCOL = meta["TOTCOL"]
    DMAX = meta["DMAX"]
    DM8 = ((DMAX + 7) // 8) * 8
    dk_scale = 1.0 / math.sqrt(H)

    nc = bacc.Bacc()
    xe = nc.declare_dram_parameter("xe", [D, TOTCOL * P], bf16, isOutput=False)
    xtq = nc.declare_dram_parameter("xtq", [D, NLOC], bf16, isOutput=False)
    w = nc.declare_dram_parameter("w", [D, 3 * H], bf16, isOutput=False)
    mask = nc.declare_dram_parameter("mask", [P, TOTCOL], f32, isOutput=False)
    out = nc.declare_dram_parameter("out", [NLOC, H], f32, isOutput=True)

    with tile.TileContext(nc) as tc, ExitStack() as ctx0:
        with tc.tile_pool(name="const", bufs=1) as cpool:
            w_sb = cpool.tile([P, DC, 3 * H], bf16)
            nc.sync.dma_start(w_sb[:], w[:].rearrange("(c p) m -> p c m", p=P))
            mask_sb = cpool.tile([P, TOTCOL], f32)
            nc.sync.dma_start(mask_sb[:], mask[:])
            qtile = cpool.tile([P, NK * H], bf16)

            # ---- Phase Q: own-node queries into resident SBUF table --------
            with tc.tile_pool(name="pq", bufs=2) as pq, \
                 tc.tile_pool(name="pq_ps", bufs=2, space="PSUM") as pq_ps:
                QB = 8
                b0 = 0
                while b0 < NK:
                    qb = min(QB, NK - b0)
                    m0 = b0 * P
                    xqb = pq.tile([P, DC, QB, P], bf16, tag="xqb")
                    for c in range(DC):
                        nc.sync.dma_start(
                            xqb[:, c, :qb, :],
                            xtq[c * P:(c + 1) * P, m0:m0 + qb * P]
                            .rearrange("p (t n) -> p t n", n=P))
                    psq = pq_ps.tile([P, QB * H], f32, tag="psQ")
                    psqv = psq[:].rearrange("p (t e) -> p t e", e=H)
                    for t in range(qb):
                        for c in range(DC):
                            nc.tensor.matmul(
                                psqv[:, t, :], lhsT=xqb[:, c, t, :],
                                rhs=w_sb[:, c, H2:3 * H],
                                start=(c == 0), stop=(c == DC - 1))
                    nc.scalar.activation(
                        qtile[:, b0 * H:(b0 + qb) * H], psq[:, :qb * H], AF.Copy)
                    b0 += qb

            # ---- Phase B: stream Xe, per-(k,j) K|V matmul, segment softmax -
            with tc.tile_pool(name="px", bufs=3) as px, \
                 tc.tile_pool(name="pps", bufs=2, space="PSUM") as pps, \
                 tc.tile_pool(name="ps1", bufs=2) as ps1, \
                 tc.tile_pool(name="ps2", bufs=2) as ps2, \
                 tc.tile_pool(name="po", bufs=2) as po:
                GB = 8                      # groups per tail batch / out DMA
                kb = 0
                while kb < NK:
                    gb = min(GB, NK - kb)
                    obuf = po.tile([P, GB * H], f32, tag="obuf")
                    nm_all = po.tile([P, GB * H], f32, tag="nm")
                    e_all = po.tile([P, GB, DM8], bf16, tag="e")
                    nc.vector.memset(e_all[:], 0.0)
                    for k in range(kb, kb + gb):
                        g = k - kb
                        d = d_sched[k]
                        cb = colbase[k]
                        wvt = ps1.tile([P, H, DM8], bf16, tag="wv")
                        qk_b = qtile[:, k * H:(k + 1) * H]
                        c0 = 0
                        while c0 < d:
                            jc = min(JC, d - c0)
                            xeb = px.tile([P, DC, JC, P], bf16, tag="xe")
                            for c in range(DC):
                                nc.sync.dma_start(
                                    xeb[:, c, :jc, :],
                                    xe[c * P:(c + 1) * P,
                                       (cb + c0) * P:(cb + c0 + jc) * P]
                                    .rearrange("p (t n) -> p t n", n=P))
                            ps = pps.tile([P, JC * H2], f32, tag="ps")
                            ps4 = ps[:].rearrange("p (t e) -> p t e", e=H2)
                            for t in range(jc):
                                for c in range(DC):
                                    nc.tensor.matmul(
                                        ps4[:, t, :], lhsT=xeb[:, c, t, :],
                                        rhs=w_sb[:, c, 0:H2],
                                        start=(c == 0), stop=(c == DC - 1))
                            kb16 = ps2.tile([P, JC, H], bf16, tag="kb16")
                            nc.scalar.activation(
                                kb16[:, :jc, :], ps4[:, :jc, 0:H], AF.Copy)
                            vb16 = ps2.tile([P, H * JC], bf16, tag="vb16")
                            vb16v = vb16[:].rearrange("p (h j) -> p j h", j=JC)
                            nc.scalar.activation(
                                vb16v[:, :jc, :], ps4[:, :jc, H:H2], AF.Copy)
                            qk = ps2.tile([P, JC, H], bf16, tag="qk")
                            nc.vector.tensor_tensor(
                                out=qk[:, :jc, :], in0=kb16[:, :jc, :],
                                in1=qk_b.unsqueeze(1).to_broadcast([P, jc, H]),
                                op=OP.mult)
                            sc = ps2.tile([P, JC], f32, tag="sc")
                            nc.vector.tensor_reduce(
                                out=sc[:, :jc], in_=qk[:, :jc, :],
                                axis=AX.X, op=OP.add)
                            sm = ps2.tile([P, JC], f32, tag="sm")
                            nc.vector.tensor_tensor(
                                out=sm[:, :jc], in0=sc[:, :jc],
                                in1=mask_sb[:, cb + c0:cb + c0 + jc], op=OP.add)
                            nc.scalar.activation(
                                e_all[:, g, c0:c0 + jc], sm[:, :jc], AF.Exp,
                                scale=dk_scale)
                            nc.vector.tensor_tensor(
                                out=wvt[:, :, c0:c0 + jc],
                                in0=vb16[:].rearrange(
                                    "p (h j) -> p h j", j=JC)[:, :, :jc],
                                in1=e_all[:, g, c0:c0 + jc].unsqueeze(1)
                                .to_broadcast([P, H, jc]),
                                op=OP.mult)
                            c0 += jc
                        nc.vector.tensor_reduce(
                            out=nm_all[:, g * H:(g + 1) * H],
                            in_=wvt[:, :, :d], axis=AX.X, op=OP.add)
                    # batched tails for the kb block
                    dn = ps2.tile([P, 3 * GB], f32, tag="dn")
                    nc.vector.tensor_reduce(
                        out=dn[:, 0:gb], in_=e_all[:, :gb, :], axis=AX.X,
                        op=OP.add)
                    nc.vector.tensor_scalar_max(
                        out=dn[:, GB:GB + gb], in0=dn[:, 0:gb], scalar1=1e-38)
                    nc.vector.reciprocal(dn[:, 2 * GB:2 * GB + gb],
                                         dn[:, GB:GB + gb])
                    nc.vector.tensor_tensor(
                        out=obuf[:, :gb * H].rearrange("p (g h) -> p g h", h=H),
                        in0=nm_all[:, :gb * H].rearrange(
                            "p (g h) -> p g h", h=H),
                        in1=dn[:, 2 * GB:2 * GB + gb].unsqueeze(2)
                        .to_broadcast([P, gb, H]),
                        op=OP.mult)
                    nc.sync.dma_start(
                        out[kb * P:(kb + gb) * P, :].rearrange(
                            "(g p) h -> p g h", p=P),
                        obuf[:, :gb * H])
                    kb += gb

    nc.finalize()
    return nc


_CACHE = {}


def _get_program(cfg, meta):
    key = (cfg["N"], cfg["D"], cfg["H"], tuple(meta["d_sched"]))
    if key not in _CACHE:
        _CACHE[key] = _build_program(cfg, meta)
    return _CACHE[key]


def run(X, Wq, Wk, Wv, edge_index, trace=False, tmpdir=None):
    from concourse.bass_utils import run_bass_kernel_spmd

    X = np.asarray(X)
    N, D = X.shape
    H = np.asarray(Wq).shape[1]
    cfg = _cfg_from_shapes(N, D, H)
    meta, in_maps, post = _prep(cfg, X, Wq, Wk, Wv, edge_index)
    nc = _get_program(cfg, meta)
    res = run_bass_kernel_spmd(
        nc, in_maps, list(range(NC)), trace=trace, tmpdir=tmpdir)

    NLOC, NDUM = cfg["NLOC"], post["NDUM"]
    order = post["order"]
    out_pos = np.empty((cfg["NPOS"], H), np.float32)
    kk = np.arange(NLOC)
    for c in range(NC):
        gpos = ((kk // P) * NC + c) * P + kk % P
        out_pos[gpos] = res.results[c]["out"]
    out_full = np.empty((N, H), np.float32)
    out_full[order] = out_pos[NDUM:]
    return out_full, res


def kernel(X, Wq, Wk, Wv, edge_index):
    out, _ = run(X, Wq, Wk, Wv, edge_index, trace=False)
    return out


# revision 8
# speedup vs baseline: 1.1578x; 1.1578x over previous
"""GNN message-passing (segment-softmax attention aggregation) on 8 TRN2 cores.

Strategy v3 (edge-expanded X streaming — no dma_gather):
- Nodes sorted by degree -> canonical positions; group g = pos//128 owned by
  core g%8.  Each core owns NLOC nodes in NK groups of 128.
- Host builds, per core, an edge-expanded table Xe[D, S] (bf16): column
  (colbase[k] + j)*128 + p  holds X[dst of edge j of node p in group k],
  padded to the common (max-over-cores) per-group degree d_sched[k].
- Device: Q for own nodes via matmul (resident bf16 in SBUF).  Main loop
  streams Xe in 4KB-per-partition slabs; per (group k, edge j) a 128x128
  LDW+MM pair computes K|V for all 128 nodes of the group in node-major PSUM
  layout.  ACT casts K|V to bf16 SBUF (V stored h-major); DVE computes q.k
  scores (bf16 2x mult + 1x reduce), ACT exp with additive -1e30 pad mask,
  DVE e*V (bf16 2x); per-group reduce over j gives num, batched tail ops per
  8 groups give out = num/max(den,eps).
- Host reassembles the full [N, H] output from the 8 per-core outputs.

Softmax max-subtraction is skipped: scores are small here, exp is safe in
fp32 and softmax is shift-invariant, so results match to fp32 rounding.
"""

import math
import sys

import ml_dtypes
import numpy as np

for _p in ("/opt/trn_rl_repo", "/root/.axon_site/_ro/trn_rl_repo"):
    if _p not in sys.path:
        sys.path.append(_p)

P = 128
NC = 8
JC = 16           # psum chunk: j-slots per batch
NEG = -1.0e30


def _cfg_from_shapes(N, D, H):
    NPOS = ((N + 1023) // 1024) * 1024
    NG = NPOS // P
    NK = NG // NC
    NLOC = NK * P
    return dict(N=N, D=D, H=H, NPOS=NPOS, NG=NG, NK=NK, NLOC=NLOC)


def _prep(cfg, X, Wq, Wk, Wv, edge_index):
    N, D, H = cfg["N"], cfg["D"], cfg["H"]
    NPOS, NK, NLOC = cfg["NPOS"], cfg["NK"], cfg["NLOC"]
    NDUM = NPOS - N
    BF = ml_dtypes.bfloat16

    src = np.asarray(edge_index[0], dtype=np.int64)
    dst = np.asarray(edge_index[1], dtype=np.int64)
    E = src.shape[0]

    deg = np.bincount(src, minlength=N)
    order = np.argsort(deg, kind="stable")          # real nodes, degree asc
    pos_of = np.empty(N, np.int64)
    pos_of[order] = NDUM + np.arange(N)

    spos = pos_of[src]
    ecore = (spos // P) % NC
    sloc = (spos // (P * NC)) * P + spos % P        # canonical-local row

    # per (core, loc) degree -> common per-group schedule
    key = ecore * NLOC + sloc
    deg_cl = np.bincount(key, minlength=NC * NLOC).reshape(NC, NLOC)
    d_sched = deg_cl.reshape(NC, NK, P).max(axis=(0, 2))
    d_sched = np.maximum(d_sched, 1)                # [NK]
    colbase = np.concatenate(([0], np.cumsum(d_sched)))[:-1]
    TOTCOL = int(d_sched.sum())
    DMAX = int(d_sched.max())

    # per-edge slot assignment (j = rank within (core, loc))
    eorder = np.argsort(key, kind="stable")
    counts = deg_cl.ravel()
    starts = np.concatenate(([0], np.cumsum(counts)))[:-1]
    j_of = np.arange(E) - starts[key[eorder]]
    se_loc = sloc[eorder]
    se_core = ecore[eorder]
    se_dst = dst[eorder]
    se_k = se_loc // P
    se_p = se_loc % P
    se_col = colbase[se_k] + j_of                   # column in [0, TOTCOL)
    se_slot = se_col * P + se_p                     # flat slot id per core

    Xet = np.ascontiguousarray(np.asarray(X, np.float32).T.astype(BF))  # [D, N]

    kk = np.arange(NLOC)
    in_maps = []
    for c in range(NC):
        m_e = se_core == c
        slots = se_slot[m_e]
        dst_slot = np.zeros(TOTCOL * P, np.int64)
        dst_slot[slots] = se_dst[m_e]
        valid = np.zeros(TOTCOL * P, bool)
        valid[slots] = True
        xe = np.ascontiguousarray(Xet[:, dst_slot])             # [D, S]
        mask = np.where(valid.reshape(TOTCOL, P).T, 0.0,
                        np.float32(NEG)).astype(np.float32)     # [128, TOTCOL]

        gpos = ((kk // P) * NC + c) * P + kk % P                # canonical pos
        node_of_loc = np.zeros(NLOC, np.int64)
        real = gpos >= NDUM
        node_of_loc[real] = order[gpos[real] - NDUM]
        xtq = np.ascontiguousarray(Xet[:, node_of_loc])         # [D, NLOC]

        in_maps.append({"xe": xe, "xtq": xtq, "mask": mask})

    w = np.concatenate(
        [np.asarray(Wk, np.float32), np.asarray(Wv, np.float32),
         np.asarray(Wq, np.float32)], axis=1).astype(BF)        # [D, 3H]
    for m in in_maps:
        m["w"] = w

    meta = dict(d_sched=d_sched.tolist(), colbase=colbase.tolist(),
                TOTCOL=TOTCOL, DMAX=DMAX)
    post = dict(order=order, NDUM=NDUM)
    return meta, in_maps, post


def _build_program(cfg, meta):
    import concourse.bass as bass
    import concourse.tile as tile
    from concourse import bacc, mybir
    from contextlib import ExitStack

    f32 = mybir.dt.float32
    bf16 = mybir.dt.bfloat16
    AF = mybir.ActivationFunctionType
    OP = mybir.AluOpType
    AX = mybir.AxisListType

    D, H = cfg["D"], cfg["H"]
    NK, NLOC = cfg["NK"], cfg["NLOC"]
    H2 = 2 * H
    DC = D // P
    d_sched = meta["d_sched"]
    colbase = meta["colbase"]
    TOTCOL = meta["TOTCOL"]
    DMAX = meta["DMAX"]
    DM8 = ((DMAX + 7) // 8) * 8
    dk_scale = 1.0 / math.sqrt(H)

    nc = bacc.Bacc()
    xe = nc.declare_dram_parameter("xe", [D, TOTCOL * P], bf16, isOutput=False)
    xtq = nc.declare_dram_parameter("xtq", [D, NLOC], bf16, isOutput=False)
    w = nc.declare_dram_parameter("w", [D, 3 * H], bf16, isOutput=False)
    mask = nc.declare_dram_parameter("mask", [P, TOTCOL], f32, isOutput=False)
    out = nc.declare_dram_parameter("out", [NLOC, H], f32, isOutput=True)

    with tile.TileContext(nc) as tc, ExitStack() as ctx0:
        with tc.tile_pool(name="const", bufs=1) as cpool:
            w_sb = cpool.tile([P, DC, 3 * H], bf16)
            nc.sync.dma_start(w_sb[:], w[:].rearrange("(c p) m -> p c m", p=P))
            mask_sb = cpool.tile([P, TOTCOL], f32)
            nc.sync.dma_start(mask_sb[:], mask[:])
            qtile = cpool.tile([P, NK * H], bf16)

            # ---- Phase Q: own-node queries into resident SBUF table --------
            with tc.tile_pool(name="pq", bufs=2) as pq, \
                 tc.tile_pool(name="pq_ps", bufs=2, space="PSUM") as pq_ps:
                QB = 8
                b0 = 0
                while b0 < NK:
                    qb = min(QB, NK - b0)
                    m0 = b0 * P
                    xqb = pq.tile([P, DC, QB, P], bf16, tag="xqb")
                    for c in range(DC):
                        nc.sync.dma_start(
                            xqb[:, c, :qb, :],
                            xtq[c * P:(c + 1) * P, m0:m0 + qb * P]
                            .rearrange("p (t n) -> p t n", n=P))
                    psq = pq_ps.tile([P, QB * H], f32, tag="psQ")
                    psqv = psq[:].rearrange("p (t e) -> p t e", e=H)
                    for t in range(qb):
                        for c in range(DC):
                            nc.tensor.matmul(
                                psqv[:, t, :], lhsT=xqb[:, c, t, :],
                                rhs=w_sb[:, c, H2:3 * H],
                                start=(c == 0), stop=(c == DC - 1))
                    nc.scalar.activation(
                        qtile[:, b0 * H:(b0 + qb) * H], psq[:, :qb * H], AF.Copy)
                    b0 += qb

            # ---- Phase B: stream Xe, per-(k,j) K|V matmul, segment softmax -
            with tc.tile_pool(name="px", bufs=3) as px, \
                 tc.tile_pool(name="pps", bufs=2, space="PSUM") as pps, \
                 tc.tile_pool(name="ps1", bufs=2) as ps1, \
                 tc.tile_pool(name="ps2", bufs=2) as ps2, \
                 tc.tile_pool(name="po", bufs=2) as po:
                GB = 8                      # groups per tail batch / out DMA
                kb = 0
                while kb < NK:
                    gb = min(GB, NK - kb)
                    obuf = po.tile([P, GB * H], f32, tag="obuf")
                    nm_all = po.tile([P, GB * H], f32, tag="nm")
                    e_all = po.tile([P, GB, DM8], bf16, tag="e")
                    nc.vector.memset(e_all[:], 0.0)
                    for k in range(kb, kb + gb):
                        g = k - kb
                        d = d_sched[k]
                        cb = colbase[k]
                        wvt = ps1.tile([P, DM8 * H], bf16, tag="wv")
                        qk_b = qtile[:, k * H:(k + 1) * H]
                        c0 = 0
                        while c0 < d:
                            jc = min(JC, d - c0)
                            xeb = px.tile([P, DC, JC, P], bf16, tag="xe")
                            for c in range(DC):
                                nc.sync.dma_start(
                                    xeb[:, c, :jc, :],
                                    xe[c * P:(c + 1) * P,
                                       (cb + c0) * P:(cb + c0 + jc) * P]
                                    .rearrange("p (t n) -> p t n", n=P))
                            ps = pps.tile([P, JC * H2], f32, tag="ps")
                            ps4 = ps[:].rearrange("p (t e) -> p t e", e=H2)
                            for t in range(jc):
                                for c in range(DC):
                                    nc.tensor.matmul(
                                        ps4[:, t, :], lhsT=xeb[:, c, t, :],
                                        rhs=w_sb[:, c, 0:H2],
                                        start=(c == 0), stop=(c == DC - 1))
                            kb16 = ps2.tile([P, JC, H], bf16, tag="kb16")
                            nc.scalar.activation(
                                kb16[:, :jc, :], ps4[:, :jc, 0:H], AF.Copy)
                            qk = ps2.tile([P, JC, H], bf16, tag="qk")
                            nc.vector.tensor_tensor(
                                out=qk[:, :jc, :], in0=kb16[:, :jc, :],
                                in1=qk_b.unsqueeze(1).to_broadcast([P, jc, H]),
                                op=OP.mult)
                            sc = ps2.tile([P, JC], f32, tag="sc")
                            nc.vector.tensor_reduce(
                                out=sc[:, :jc], in_=qk[:, :jc, :],
                                axis=AX.X, op=OP.add)
                            sm = ps2.tile([P, JC], f32, tag="sm")
                            nc.vector.tensor_tensor(
                                out=sm[:, :jc], in0=sc[:, :jc],
                                in1=mask_sb[:, cb + c0:cb + c0 + jc], op=OP.add)
                            nc.scalar.activation(
                                e_all[:, g, c0:c0 + jc], sm[:, :jc], AF.Exp,
                                scale=dk_scale)
                            nc.vector.tensor_tensor(
                                out=wvt[:].rearrange(
                                    "p (j h) -> p j h", h=H)[:, c0:c0 + jc, :],
                                in0=ps4[:, :jc, H:H2],
                                in1=e_all[:, g, c0:c0 + jc].unsqueeze(2)
                                .to_broadcast([P, jc, H]),
                                op=OP.mult)
                            c0 += jc
                        nc.vector.tensor_reduce(
                            out=nm_all[:, g * H:(g + 1) * H],
                            in_=wvt[:].rearrange(
                                "p (j h) -> p h j", h=H)[:, :, :d],
                            axis=AX.X, op=OP.add)
                    # batched tails for the kb block
                    dn = ps2.tile([P, 3 * GB], f32, tag="dn")
                    nc.vector.tensor_reduce(
                        out=dn[:, 0:gb], in_=e_all[:, :gb, :], axis=AX.X,
                        op=OP.add)
                    nc.vector.tensor_scalar_max(
                        out=dn[:, GB:GB + gb], in0=dn[:, 0:gb], scalar1=1e-38)
                    nc.vector.reciprocal(dn[:, 2 * GB:2 * GB + gb],
                                         dn[:, GB:GB + gb])
                    nc.vector.tensor_tensor(
                        out=obuf[:, :gb * H].rearrange("p (g h) -> p g h", h=H),
                        in0=nm_all[:, :gb * H].rearrange(
                            "p (g h) -> p g h", h=H),
                        in1=dn[:, 2 * GB:2 * GB + gb].unsqueeze(2)
                        .to_broadcast([P, gb, H]),
                        op=OP.mult)
                    nc.sync.dma_start(
                        out[kb * P:(kb + gb) * P, :].rearrange(
                            "(g p) h -> p g h", p=P),
                        obuf[:, :gb * H])
                    kb += gb

    nc.finalize()
    return nc


_CACHE = {}


def _get_program(cfg, meta):
    key = (cfg["N"], cfg["D"], cfg["H"], tuple(meta["d_sched"]))
    if key not in _CACHE:
        _CACHE[key] = _build_program(cfg, meta)
    return _CACHE[key]


def run(X, Wq, Wk, Wv, edge_index, trace=False, tmpdir=None):
    from concourse.bass_utils import run_bass_kernel_spmd

    X = np.asarray(X)
    N, D = X.shape
    H = np.asarray(Wq).shape[1]
    cfg = _cfg_from_shapes(N, D, H)
    meta, in_maps, post = _prep(cfg, X, Wq, Wk, Wv, edge_index)
    nc = _get_program(cfg, meta)
    res = run_bass_kernel_spmd(
        nc, in_maps, list(range(NC)), trace=trace, tmpdir=tmpdir)

    NLOC, NDUM = cfg["NLOC"], post["NDUM"]
    order = post["order"]
    out_pos = np.empty((cfg["NPOS"], H), np.float32)
    kk = np.arange(NLOC)
    for c in range(NC):
        gpos = ((kk // P) * NC + c) * P + kk % P
        out_pos[gpos] = res.results[c]["out"]
    out_full = np.empty((N, H), np.float32)
    out_full[order] = out_pos[NDUM:]
    return out_full, res


def kernel(X, Wq, Wk, Wv, edge_index):
    out, _ = run(X, Wq, Wk, Wv, edge_index, trace=False)
    return out


# revision 11
# speedup vs baseline: 1.1874x; 1.0256x over previous
"""GNN message-passing (segment-softmax attention aggregation) on 8 TRN2 cores.

Strategy v5 (edge-expanded X streaming — no dma_gather, no mask):
- Nodes sorted by degree -> canonical positions; group g = pos//128 owned by
  core g%8.  Each core owns NLOC nodes in NK groups of 128.
- Groups are packed into batches (G consecutive groups padded to a common
  degree db, G*db <= 16 columns, or G=1 for db>16).  Host builds, per core,
  an edge-expanded table Xe[D, S] (bf16): column (cb + g*db + j)*128 + p
  holds X[dst of edge j of node p of the batch's g-th group]; padding
  columns are ZERO vectors, so K=V=0 and exp(score)=1 there — the known
  per-node pad count is subtracted from the softmax denominator instead of
  an additive mask.
- Device: Q via matmul (resident bf16 SBUF).  Main loop streams Xe in
  4KB-per-partition slabs; per slot-column a 128x128 LDW+MM pair computes
  K|V for 128 nodes in node-major PSUM.  ACT casts K to bf16 (qk mult runs
  2x), DVE does scores/e*V/reduces, ACT exp; batched per-batch tails give
  out = num/max(den - padcnt, eps).
- Host reassembles the full [N, H] output from the 8 per-core outputs.

Softmax max-subtraction is skipped: scores are small here, exp is safe in
fp32 and softmax is shift-invariant, so results match to fp32 rounding.
"""

import math
import sys

import ml_dtypes
import numpy as np

for _p in ("/opt/trn_rl_repo", "/root/.axon_site/_ro/trn_rl_repo"):
    if _p not in sys.path:
        sys.path.append(_p)

P = 128
NC = 8
JC = 16           # psum chunk: slot columns per batch chunk
NEG = -1.0e30


def _cfg_from_shapes(N, D, H):
    NPOS = ((N + 1023) // 1024) * 1024
    NG = NPOS // P
    NK = NG // NC
    NLOC = NK * P
    return dict(N=N, D=D, H=H, NPOS=NPOS, NG=NG, NK=NK, NLOC=NLOC)


def _make_batches(d_sched):
    """Pack degree-sorted groups: (k0, G, db, cb) with G*db <= JC or G=1."""
    NK = len(d_sched)
    batches = []
    cb = 0
    k = 0
    while k < NK:
        G = 1
        while (
            k + G < NK
            and (G + 1) * d_sched[k + G] <= JC
        ):
            G += 1
        db = int(d_sched[k + G - 1])
        batches.append((k, G, db, cb))
        cb += G * db
        k += G
    return batches, cb


def _prep(cfg, X, Wq, Wk, Wv, edge_index):
    N, D, H = cfg["N"], cfg["D"], cfg["H"]
    NPOS, NK, NLOC = cfg["NPOS"], cfg["NK"], cfg["NLOC"]
    NDUM = NPOS - N
    BF = ml_dtypes.bfloat16

    src = np.asarray(edge_index[0], dtype=np.int64)
    dst = np.asarray(edge_index[1], dtype=np.int64)
    E = src.shape[0]

    deg = np.bincount(src, minlength=N)
    order = np.argsort(deg, kind="stable")          # real nodes, degree asc
    pos_of = np.empty(N, np.int64)
    pos_of[order] = NDUM + np.arange(N)

    spos = pos_of[src]
    ecore = (spos // P) % NC
    sloc = (spos // (P * NC)) * P + spos % P        # canonical-local row

    # per (core, loc) degree -> common per-group schedule
    key = ecore * NLOC + sloc
    deg_cl = np.bincount(key, minlength=NC * NLOC).reshape(NC, NLOC)
    d_sched = deg_cl.reshape(NC, NK, P).max(axis=(0, 2))
    d_sched = np.maximum(d_sched, 1)                # [NK]

    batches, TOTCOL = _make_batches(d_sched.tolist())
    DMAX = max(db for (_, _, db, _) in batches)
    # per-group column base and padded degree
    colbase_g = np.zeros(NK, np.int64)
    dpad_g = np.zeros(NK, np.int64)
    for (k0, G, db, cb) in batches:
        for g in range(G):
            colbase_g[k0 + g] = cb + g * db
            dpad_g[k0 + g] = db

    # per-edge slot assignment (j = rank within (core, loc))
    eorder = np.argsort(key, kind="stable")
    counts = deg_cl.ravel()
    starts = np.concatenate(([0], np.cumsum(counts)))[:-1]
    j_of = np.arange(E) - starts[key[eorder]]
    se_loc = sloc[eorder]
    se_core = ecore[eorder]
    se_dst = dst[eorder]
    se_k = se_loc // P
    se_p = se_loc % P
    se_col = colbase_g[se_k] + j_of                 # column in [0, TOTCOL)
    se_slot = se_col * P + se_p                     # flat slot id per core

    Xet = np.ascontiguousarray(np.asarray(X, np.float32).T.astype(BF))  # [D, N]

    kk = np.arange(NLOC)
    in_maps = []
    for c in range(NC):
        m_e = se_core == c
        slots = se_slot[m_e]
        dst_slot = np.full(TOTCOL * P, -1, np.int64)
        dst_slot[slots] = se_dst[m_e]
        pad = dst_slot < 0
        dst_slot[pad] = 0
        xe = Xet[:, dst_slot]                                   # [D, S]
        xe[:, pad] = BF(0.0)
        xe = np.ascontiguousarray(xe)
        # padcnt[p, k] = dpad_k - deg(core, loc=k*128+p)
        padcnt = (dpad_g[None, :] -
                  deg_cl[c].reshape(NK, P).T).astype(np.float32)  # [128, NK]

        gpos = ((kk // P) * NC + c) * P + kk % P                # canonical pos
        node_of_loc = np.zeros(NLOC, np.int64)
        real = gpos >= NDUM
        node_of_loc[real] = order[gpos[real] - NDUM]
        xtq = np.ascontiguousarray(Xet[:, node_of_loc])         # [D, NLOC]

        in_maps.append({"xe": xe, "xtq": xtq, "padcnt": padcnt})

    w = np.concatenate(
        [np.asarray(Wk, np.float32), np.asarray(Wv, np.float32),
         np.asarray(Wq, np.float32)], axis=1).astype(BF)        # [D, 3H]
    for m in in_maps:
        m["w"] = w

    meta = dict(batches=batches, TOTCOL=TOTCOL, DMAX=DMAX)
    post = dict(order=order, NDUM=NDUM)
    return meta, in_maps, post


def _build_program(cfg, meta):
    import concourse.bass as bass
    import concourse.tile as tile
    from concourse import bacc, mybir
    from contextlib import ExitStack

    f32 = mybir.dt.float32
    bf16 = mybir.dt.bfloat16
    AF = mybir.ActivationFunctionType
    OP = mybir.AluOpType
    AX = mybir.AxisListType

    D, H = cfg["D"], cfg["H"]
    NK, NLOC = cfg["NK"], cfg["NLOC"]
    H2 = 2 * H
    DC = D // P
    batches = meta["batches"]
    TOTCOL = meta["TOTCOL"]
    DMAX = meta["DMAX"]
    EW = max(DMAX, JC)
    dk_scale = 1.0 / math.sqrt(H)

    nc = bacc.Bacc()
    xe = nc.declare_dram_parameter("xe", [D, TOTCOL * P], bf16, isOutput=False)
    xtq = nc.declare_dram_parameter("xtq", [D, NLOC], bf16, isOutput=False)
    w = nc.declare_dram_parameter("w", [D, 3 * H], bf16, isOutput=False)
    padcnt = nc.declare_dram_parameter("padcnt", [P, NK], f32, isOutput=False)
    out = nc.declare_dram_parameter("out", [NLOC, H], f32, isOutput=True)

    with tile.TileContext(nc) as tc, ExitStack() as ctx0:
        with tc.tile_pool(name="const", bufs=1) as cpool:
            w_sb = cpool.tile([P, DC, 3 * H], bf16)
            nc.sync.dma_start(w_sb[:], w[:].rearrange("(c p) m -> p c m", p=P))
            pc_sb = cpool.tile([P, NK], f32)
            nc.sync.dma_start(pc_sb[:], padcnt[:])
            qtile = cpool.tile([P, NK * H], bf16)

            # ---- Phase Q: own-node queries into resident SBUF table --------
            with tc.tile_pool(name="pq", bufs=2) as pq, \
                 tc.tile_pool(name="pq_ps", bufs=2, space="PSUM") as pq_ps:
                QB = 8
                b0 = 0
                while b0 < NK:
                    qb = min(QB, NK - b0)
                    m0 = b0 * P
                    xqb = pq.tile([P, DC, QB, P], bf16, tag="xqb")
                    for c in range(DC):
                        nc.sync.dma_start(
                            xqb[:, c, :qb, :],
                            xtq[c * P:(c + 1) * P, m0:m0 + qb * P]
                            .rearrange("p (t n) -> p t n", n=P))
                    psq = pq_ps.tile([P, QB * H], f32, tag="psQ")
                    psqv = psq[:].rearrange("p (t e) -> p t e", e=H)
                    for t in range(qb):
                        for c in range(DC):
                            nc.tensor.matmul(
                                psqv[:, t, :], lhsT=xqb[:, c, t, :],
                                rhs=w_sb[:, c, H2:3 * H],
                                start=(c == 0), stop=(c == DC - 1))
                    nc.scalar.activation(
                        qtile[:, b0 * H:(b0 + qb) * H], psq[:, :qb * H], AF.Copy)
                    b0 += qb

            # ---- Phase B: stream Xe, K|V matmuls, segment softmax ----------
            with tc.tile_pool(name="px", bufs=3) as px, \
                 tc.tile_pool(name="pps", bufs=2, space="PSUM") as pps, \
                 tc.tile_pool(name="ps1", bufs=2) as ps1, \
                 tc.tile_pool(name="ps2", bufs=3) as ps2, \
                 tc.tile_pool(name="po", bufs=2) as po:
                for (k0, G, db, cb) in batches:
                    W = G * db
                    wv = ps1.tile([P, EW * H], bf16, tag="wv")
                    wv4 = wv[:].rearrange("p (j h) -> p j h", h=H)
                    e_b = ps1.tile([P, EW], bf16, tag="e")
                    c0 = 0
                    while c0 < W:
                        jc = min(JC, W - c0)
                        xeb = px.tile([P, DC, JC, P], bf16, tag="xe")
                        for c in range(DC):
                            nc.sync.dma_start(
                                xeb[:, c, :jc, :],
                                xe[c * P:(c + 1) * P,
                                   (cb + c0) * P:(cb + c0 + jc) * P]
                                .rearrange("p (t n) -> p t n", n=P))
                        ps = pps.tile([P, JC * H2], f32, tag="ps")
                        ps4 = ps[:].rearrange("p (t e) -> p t e", e=H2)
                        for t in range(jc):
                            for c in range(DC):
                                nc.tensor.matmul(
                                    ps4[:, t, :], lhsT=xeb[:, c, t, :],
                                    rhs=w_sb[:, c, 0:H2],
                                    start=(c == 0), stop=(c == DC - 1))
                        kb16 = ps2.tile([P, JC, H], bf16, tag="kb16")
                        nc.scalar.activation(
                            kb16[:, :jc, :], ps4[:, :jc, 0:H], AF.Copy)
                        qk = ps2.tile([P, JC, H], bf16, tag="qk")
                        sc = ps2.tile([P, JC], f32, tag="sc")
                        if G > 1:
                            nc.vector.tensor_tensor(
                                out=qk[:, :W, :].rearrange(
                                    "p (g j) h -> p g j h", g=G),
                                in0=kb16[:, :W, :].rearrange(
                                    "p (g j) h -> p g j h", g=G),
                                in1=qtile[:, k0 * H:(k0 + G) * H]
                                .rearrange("p (g h) -> p g h", h=H)
                                .unsqueeze(2).to_broadcast([P, G, db, H]),
                                op=OP.mult)
                            nc.vector.tensor_reduce(
                                out=sc[:, :W].rearrange(
                                    "p (g j) -> p g j", g=G),
                                in_=qk[:, :W, :].rearrange(
                                    "p (g j) h -> p g j h", g=G),
                                axis=AX.X, op=OP.add)
                        else:
                            nc.vector.tensor_tensor(
                                out=qk[:, :jc, :], in0=kb16[:, :jc, :],
                                in1=qtile[:, k0 * H:(k0 + 1) * H]
                                .unsqueeze(1).to_broadcast([P, jc, H]),
                                op=OP.mult)
                            nc.vector.tensor_reduce(
                                out=sc[:, :jc], in_=qk[:, :jc, :],
                                axis=AX.X, op=OP.add)
                        nc.scalar.activation(
                            e_b[:, c0:c0 + jc], sc[:, :jc], AF.Exp,
                            scale=dk_scale)
                        nc.vector.tensor_tensor(
                            out=wv4[:, c0:c0 + jc, :],
                            in0=ps4[:, :jc, H:H2],
                            in1=e_b[:, c0:c0 + jc].unsqueeze(2)
                            .to_broadcast([P, jc, H]),
                            op=OP.mult)
                        c0 += jc
                    # ---- batched batch tail --------------------------------
                    nm = ps2.tile([P, JC * H], f32, tag="nm")
                    nc.vector.tensor_reduce(
                        out=nm[:, :G * H].rearrange("p (g h) -> p g h", h=H),
                        in_=wv[:, :W * H].rearrange(
                            "p (g j h) -> p g h j", g=G, h=H),
                        axis=AX.X, op=OP.add)
                    dn = ps2.tile([P, 4 * JC], f32, tag="dn")
                    nc.vector.tensor_reduce(
                        out=dn[:, 0:G],
                        in_=e_b[:, :W].rearrange("p (g j) -> p g j", g=G),
                        axis=AX.X, op=OP.add)
                    nc.vector.tensor_tensor(
                        out=dn[:, JC:JC + G], in0=dn[:, 0:G],
                        in1=pc_sb[:, k0:k0 + G], op=OP.subtract)
                    nc.vector.tensor_scalar_max(
                        out=dn[:, 2 * JC:2 * JC + G], in0=dn[:, JC:JC + G],
                        scalar1=1e-38)
                    nc.vector.reciprocal(dn[:, 3 * JC:3 * JC + G],
                                         dn[:, 2 * JC:2 * JC + G])
                    obuf = po.tile([P, JC * H], f32, tag="obuf")
                    nc.vector.tensor_tensor(
                        out=obuf[:, :G * H].rearrange("p (g h) -> p g h", h=H),
                        in0=nm[:, :G * H].rearrange("p (g h) -> p g h", h=H),
                        in1=dn[:, 3 * JC:3 * JC + G].unsqueeze(2)
                        .to_broadcast([P, G, H]),
                        op=OP.mult)
                    nc.sync.dma_start(
                        out[k0 * P:(k0 + G) * P, :].rearrange(
                            "(g p) h -> p g h", p=P),
                        obuf[:, :G * H])

    nc.finalize()
    return nc


_CACHE = {}


def _get_program(cfg, meta):
    key = (cfg["N"], cfg["D"], cfg["H"],
           tuple((a, b, c) for (a, b, c, _) in meta["batches"]))
    if key not in _CACHE:
        _CACHE[key] = _build_program(cfg, meta)
    return _CACHE[key]


def run(X, Wq, Wk, Wv, edge_index, trace=False, tmpdir=None):
    from concourse.bass_utils import run_bass_kernel_spmd

    X = np.asarray(X)
    N, D = X.shape
    H = np.asarray(Wq).shape[1]
    cfg = _cfg_from_shapes(N, D, H)
    meta, in_maps, post = _prep(cfg, X, Wq, Wk, Wv, edge_index)
    nc = _get_program(cfg, meta)
    res = run_bass_kernel_spmd(
        nc, in_maps, list(range(NC)), trace=trace, tmpdir=tmpdir)

    NLOC, NDUM = cfg["NLOC"], post["NDUM"]
    order = post["order"]
    out_pos = np.empty((cfg["NPOS"], H), np.float32)
    kk = np.arange(NLOC)
    for c in range(NC):
        gpos = ((kk // P) * NC + c) * P + kk % P
        out_pos[gpos] = res.results[c]["out"]
    out_full = np.empty((N, H), np.float32)
    out_full[order] = out_pos[NDUM:]
    return out_full, res


def kernel(X, Wq, Wk, Wv, edge_index):
    out, _ = run(X, Wq, Wk, Wv, edge_index, trace=False)
    return out


# revision 12
# speedup vs baseline: 1.5256x; 1.2848x over previous
"""GNN message-passing (segment-softmax attention aggregation) on 8 TRN2 cores.

Strategy v5 (edge-expanded X streaming — no dma_gather, no mask):
- Nodes sorted by degree -> canonical positions; group g = pos//128 owned by
  core g%8.  Each core owns NLOC nodes in NK groups of 128.
- Groups are packed into batches (G consecutive groups padded to a common
  degree db, G*db <= 16 columns, or G=1 for db>16).  Host builds, per core,
  an edge-expanded table Xe[D, S] (bf16): column (cb + g*db + j)*128 + p
  holds X[dst of edge j of node p of the batch's g-th group]; padding
  columns are ZERO vectors, so K=V=0 and exp(score)=1 there — the known
  per-node pad count is subtracted from the softmax denominator instead of
  an additive mask.
- Device: Q via matmul (resident bf16 SBUF).  Main loop streams Xe in
  4KB-per-partition slabs; per slot-column a 128x128 LDW+MM pair computes
  K|V for 128 nodes in node-major PSUM.  ACT casts K to bf16 (qk mult runs
  2x), DVE does scores/e*V/reduces, ACT exp; batched per-batch tails give
  out = num/max(den - padcnt, eps).
- Host reassembles the full [N, H] output from the 8 per-core outputs.

Softmax max-subtraction is skipped: scores are small here, exp is safe in
fp32 and softmax is shift-invariant, so results match to fp32 rounding.
"""

import math
import sys

import ml_dtypes
import numpy as np

for _p in ("/opt/trn_rl_repo", "/root/.axon_site/_ro/trn_rl_repo"):
    if _p not in sys.path:
        sys.path.append(_p)

P = 128
NC = 8
JC = 16           # psum chunk: slot columns per batch chunk
NEG = -1.0e30


def _cfg_from_shapes(N, D, H):
    NPOS = ((N + 1023) // 1024) * 1024
    NG = NPOS // P
    NK = NG // NC
    NLOC = NK * P
    return dict(N=N, D=D, H=H, NPOS=NPOS, NG=NG, NK=NK, NLOC=NLOC)


def _make_batches(d_sched):
    """Pack degree-sorted groups: (k0, G, db, cb) with G*db <= JC or G=1."""
    NK = len(d_sched)
    batches = []
    cb = 0
    k = 0
    while k < NK:
        G = 1
        while (
            k + G < NK
            and (G + 1) * d_sched[k + G] <= JC
        ):
            G += 1
        db = int(d_sched[k + G - 1])
        batches.append((k, G, db, cb))
        cb += G * db
        k += G
    return batches, cb


def _prep(cfg, X, Wq, Wk, Wv, edge_index):
    N, D, H = cfg["N"], cfg["D"], cfg["H"]
    NPOS, NK, NLOC = cfg["NPOS"], cfg["NK"], cfg["NLOC"]
    NDUM = NPOS - N
    BF = ml_dtypes.bfloat16

    src = np.asarray(edge_index[0], dtype=np.int64)
    dst = np.asarray(edge_index[1], dtype=np.int64)
    E = src.shape[0]

    deg = np.bincount(src, minlength=N)
    order = np.argsort(deg, kind="stable")          # real nodes, degree asc
    pos_of = np.empty(N, np.int64)
    pos_of[order] = NDUM + np.arange(N)

    spos = pos_of[src]
    ecore = (spos // P) % NC
    sloc = (spos // (P * NC)) * P + spos % P        # canonical-local row

    # per (core, loc) degree -> common per-group schedule
    key = ecore * NLOC + sloc
    deg_cl = np.bincount(key, minlength=NC * NLOC).reshape(NC, NLOC)
    d_sched = deg_cl.reshape(NC, NK, P).max(axis=(0, 2))
    d_sched = np.maximum(d_sched, 1)                # [NK]

    batches, TOTCOL = _make_batches(d_sched.tolist())
    DMAX = max(db for (_, _, db, _) in batches)
    # per-group column base and padded degree
    colbase_g = np.zeros(NK, np.int64)
    dpad_g = np.zeros(NK, np.int64)
    for (k0, G, db, cb) in batches:
        for g in range(G):
            colbase_g[k0 + g] = cb + g * db
            dpad_g[k0 + g] = db

    # per-edge slot assignment (j = rank within (core, loc))
    eorder = np.argsort(key, kind="stable")
    counts = deg_cl.ravel()
    starts = np.concatenate(([0], np.cumsum(counts)))[:-1]
    j_of = np.arange(E) - starts[key[eorder]]
    se_loc = sloc[eorder]
    se_core = ecore[eorder]
    se_dst = dst[eorder]
    se_k = se_loc // P
    se_p = se_loc % P
    se_col = colbase_g[se_k] + j_of                 # column in [0, TOTCOL)
    se_slot = se_col * P + se_p                     # flat slot id per core

    Xet = np.ascontiguousarray(np.asarray(X, np.float32).T.astype(BF))  # [D, N]

    kk = np.arange(NLOC)
    in_maps = []
    for c in range(NC):
        m_e = se_core == c
        slots = se_slot[m_e]
        dst_slot = np.full(TOTCOL * P, -1, np.int64)
        dst_slot[slots] = se_dst[m_e]
        pad = dst_slot < 0
        dst_slot[pad] = 0
        xe = Xet[:, dst_slot]                                   # [D, S]
        xe[:, pad] = BF(0.0)
        xe = np.ascontiguousarray(xe)
        # padcnt[p, k] = dpad_k - deg(core, loc=k*128+p)
        padcnt = (dpad_g[None, :] -
                  deg_cl[c].reshape(NK, P).T).astype(np.float32)  # [128, NK]

        gpos = ((kk // P) * NC + c) * P + kk % P                # canonical pos
        node_of_loc = np.zeros(NLOC, np.int64)
        real = gpos >= NDUM
        node_of_loc[real] = order[gpos[real] - NDUM]
        xtq = np.ascontiguousarray(Xet[:, node_of_loc])         # [D, NLOC]

        in_maps.append({"xe": xe, "xtq": xtq, "padcnt": padcnt})

    w = np.concatenate(
        [np.asarray(Wk, np.float32), np.asarray(Wv, np.float32),
         np.asarray(Wq, np.float32)], axis=1).astype(BF)        # [D, 3H]
    for m in in_maps:
        m["w"] = w

    meta = dict(batches=batches, TOTCOL=TOTCOL, DMAX=DMAX)
    post = dict(order=order, NDUM=NDUM)
    return meta, in_maps, post


def _build_program(cfg, meta):
    import concourse.bass as bass
    import concourse.tile as tile
    from concourse import bacc, mybir
    from contextlib import ExitStack

    f32 = mybir.dt.float32
    bf16 = mybir.dt.bfloat16
    AF = mybir.ActivationFunctionType
    OP = mybir.AluOpType
    AX = mybir.AxisListType

    D, H = cfg["D"], cfg["H"]
    NK, NLOC = cfg["NK"], cfg["NLOC"]
    H2 = 2 * H
    DC = D // P
    batches = meta["batches"]
    TOTCOL = meta["TOTCOL"]
    DMAX = meta["DMAX"]
    EW = max(DMAX, JC)
    dk_scale = 1.0 / math.sqrt(H)

    nc = bacc.Bacc()
    xe = nc.declare_dram_parameter("xe", [D, TOTCOL * P], bf16, isOutput=False)
    xtq = nc.declare_dram_parameter("xtq", [D, NLOC], bf16, isOutput=False)
    w = nc.declare_dram_parameter("w", [D, 3 * H], bf16, isOutput=False)
    padcnt = nc.declare_dram_parameter("padcnt", [P, NK], f32, isOutput=False)
    out = nc.declare_dram_parameter("out", [NLOC, H], f32, isOutput=True)

    with tile.TileContext(nc) as tc, ExitStack() as ctx0:
        with tc.tile_pool(name="const", bufs=1) as cpool:
            w_sb = cpool.tile([P, DC, 3 * H], bf16)
            nc.sync.dma_start(w_sb[:], w[:].rearrange("(c p) m -> p c m", p=P))
            pc_sb = cpool.tile([P, NK], f32)
            nc.sync.dma_start(pc_sb[:], padcnt[:])
            qtile = cpool.tile([P, NK * H], bf16)

            # ---- Phase Q: own-node queries into resident SBUF table --------
            with tc.tile_pool(name="pq", bufs=2) as pq, \
                 tc.tile_pool(name="pq_ps", bufs=2, space="PSUM") as pq_ps:
                QB = 8
                b0 = 0
                while b0 < NK:
                    qb = min(QB, NK - b0)
                    m0 = b0 * P
                    xqb = pq.tile([P, DC, QB, P], bf16, tag="xqb")
                    for c in range(DC):
                        nc.sync.dma_start(
                            xqb[:, c, :qb, :],
                            xtq[c * P:(c + 1) * P, m0:m0 + qb * P]
                            .rearrange("p (t n) -> p t n", n=P))
                    psq = pq_ps.tile([P, QB * H], f32, tag="psQ")
                    psqv = psq[:].rearrange("p (t e) -> p t e", e=H)
                    for t in range(qb):
                        for c in range(DC):
                            nc.tensor.matmul(
                                psqv[:, t, :], lhsT=xqb[:, c, t, :],
                                rhs=w_sb[:, c, H2:3 * H],
                                start=(c == 0), stop=(c == DC - 1))
                    nc.scalar.activation(
                        qtile[:, b0 * H:(b0 + qb) * H], psq[:, :qb * H], AF.Copy)
                    b0 += qb

            # ---- Phase B: stream Xe, K|V matmuls, segment softmax ----------
            # block batches into tail-blocks of >= GBMIN groups
            blocks = []
            cur = []
            gcnt = 0
            GBMIN = 8
            for b in batches:
                cur.append(b)
                gcnt += b[1]
                if gcnt >= GBMIN:
                    blocks.append(cur)
                    cur = []
                    gcnt = 0
            if cur:
                blocks.append(cur)
            GBMAX = max(sum(b[1] for b in blk) for blk in blocks)

            with tc.tile_pool(name="px", bufs=3) as px, \
                 tc.tile_pool(name="pps", bufs=2, space="PSUM") as pps, \
                 tc.tile_pool(name="ps1", bufs=2) as ps1, \
                 tc.tile_pool(name="ps2", bufs=3) as ps2, \
                 tc.tile_pool(name="po", bufs=2) as po:
                for blk in blocks:
                    bk0 = blk[0][0]
                    bgs = sum(b[1] for b in blk)
                    nm_all = po.tile([P, GBMAX * H], f32, tag="nm")
                    dn_all = po.tile([P, 4 * GBMAX], f32, tag="dn")
                    gacc = 0
                    for (k0, G, db, cb) in blk:
                        W = G * db
                        wv = ps1.tile([P, EW * H], bf16, tag="wv")
                        wv4 = wv[:].rearrange("p (j h) -> p j h", h=H)
                        e_b = ps1.tile([P, EW], bf16, tag="e")
                        c0 = 0
                        while c0 < W:
                            jc = min(JC, W - c0)
                            xeb = px.tile([P, DC, JC, P], bf16, tag="xe")
                            for c in range(DC):
                                nc.sync.dma_start(
                                    xeb[:, c, :jc, :],
                                    xe[c * P:(c + 1) * P,
                                       (cb + c0) * P:(cb + c0 + jc) * P]
                                    .rearrange("p (t n) -> p t n", n=P))
                            ps = pps.tile([P, JC * H2], f32, tag="ps")
                            ps4 = ps[:].rearrange("p (t e) -> p t e", e=H2)
                            for t in range(jc):
                                for c in range(DC):
                                    nc.tensor.matmul(
                                        ps4[:, t, :], lhsT=xeb[:, c, t, :],
                                        rhs=w_sb[:, c, 0:H2],
                                        start=(c == 0), stop=(c == DC - 1))
                            kb16 = ps2.tile([P, JC, H], bf16, tag="kb16")
                            nc.scalar.activation(
                                kb16[:, :jc, :], ps4[:, :jc, 0:H], AF.Copy)
                            vb16 = ps2.tile([P, JC, H], bf16, tag="vb16")
                            nc.scalar.activation(
                                vb16[:, :jc, :], ps4[:, :jc, H:H2], AF.Copy)
                            qk = ps2.tile([P, JC, H], bf16, tag="qk")
                            sc = ps2.tile([P, JC], f32, tag="sc")
                            if G > 1:
                                nc.vector.tensor_tensor(
                                    out=qk[:, :W, :].rearrange(
                                        "p (g j) h -> p g j h", g=G),
                                    in0=kb16[:, :W, :].rearrange(
                                        "p (g j) h -> p g j h", g=G),
                                    in1=qtile[:, k0 * H:(k0 + G) * H]
                                    .rearrange("p (g h) -> p g h", h=H)
                                    .unsqueeze(2).to_broadcast([P, G, db, H]),
                                    op=OP.mult)
                                nc.vector.tensor_reduce(
                                    out=sc[:, :W].rearrange(
                                        "p (g j) -> p g j", g=G),
                                    in_=qk[:, :W, :].rearrange(
                                        "p (g j) h -> p g j h", g=G),
                                    axis=AX.X, op=OP.add)
                            else:
                                nc.vector.tensor_tensor(
                                    out=qk[:, :jc, :], in0=kb16[:, :jc, :],
                                    in1=qtile[:, k0 * H:(k0 + 1) * H]
                                    .unsqueeze(1).to_broadcast([P, jc, H]),
                                    op=OP.mult)
                                nc.vector.tensor_reduce(
                                    out=sc[:, :jc], in_=qk[:, :jc, :],
                                    axis=AX.X, op=OP.add)
                            nc.scalar.activation(
                                e_b[:, c0:c0 + jc], sc[:, :jc], AF.Exp,
                                scale=dk_scale)
                            nc.gpsimd.tensor_tensor(
                                out=wv4[:, c0:c0 + jc, :],
                                in0=vb16[:, :jc, :],
                                in1=e_b[:, c0:c0 + jc].unsqueeze(2)
                                .to_broadcast([P, jc, H]),
                                op=OP.mult)
                            c0 += jc
                        nc.vector.tensor_reduce(
                            out=nm_all[:, gacc * H:(gacc + G) * H]
                            .rearrange("p (g h) -> p g h", h=H),
                            in_=wv[:, :W * H].rearrange(
                                "p (g j h) -> p g h j", g=G, h=H),
                            axis=AX.X, op=OP.add)
                        nc.vector.tensor_reduce(
                            out=dn_all[:, gacc:gacc + G],
                            in_=e_b[:, :W].rearrange("p (g j) -> p g j", g=G),
                            axis=AX.X, op=OP.add)
                        gacc += G
                    # ---- block tail ----------------------------------------
                    nc.vector.tensor_tensor(
                        out=dn_all[:, GBMAX:GBMAX + bgs],
                        in0=dn_all[:, 0:bgs],
                        in1=pc_sb[:, bk0:bk0 + bgs], op=OP.subtract)
                    nc.vector.tensor_scalar_max(
                        out=dn_all[:, 2 * GBMAX:2 * GBMAX + bgs],
                        in0=dn_all[:, GBMAX:GBMAX + bgs], scalar1=1e-38)
                    nc.vector.reciprocal(
                        dn_all[:, 3 * GBMAX:3 * GBMAX + bgs],
                        dn_all[:, 2 * GBMAX:2 * GBMAX + bgs])
                    obuf = po.tile([P, GBMAX * H], f32, tag="obuf")
                    nc.vector.tensor_tensor(
                        out=obuf[:, :bgs * H].rearrange(
                            "p (g h) -> p g h", h=H),
                        in0=nm_all[:, :bgs * H].rearrange(
                            "p (g h) -> p g h", h=H),
                        in1=dn_all[:, 3 * GBMAX:3 * GBMAX + bgs].unsqueeze(2)
                        .to_broadcast([P, bgs, H]),
                        op=OP.mult)
                    nc.sync.dma_start(
                        out[bk0 * P:(bk0 + bgs) * P, :].rearrange(
                            "(g p) h -> p g h", p=P),
                        obuf[:, :bgs * H])

    nc.finalize()
    return nc


_CACHE = {}


def _get_program(cfg, meta):
    key = (cfg["N"], cfg["D"], cfg["H"],
           tuple((a, b, c) for (a, b, c, _) in meta["batches"]))
    if key not in _CACHE:
        _CACHE[key] = _build_program(cfg, meta)
    return _CACHE[key]


def run(X, Wq, Wk, Wv, edge_index, trace=False, tmpdir=None):
    from concourse.bass_utils import run_bass_kernel_spmd

    X = np.asarray(X)
    N, D = X.shape
    H = np.asarray(Wq).shape[1]
    cfg = _cfg_from_shapes(N, D, H)
    meta, in_maps, post = _prep(cfg, X, Wq, Wk, Wv, edge_index)
    nc = _get_program(cfg, meta)
    res = run_bass_kernel_spmd(
        nc, in_maps, list(range(NC)), trace=trace, tmpdir=tmpdir)

    NLOC, NDUM = cfg["NLOC"], post["NDUM"]
    order = post["order"]
    out_pos = np.empty((cfg["NPOS"], H), np.float32)
    kk = np.arange(NLOC)
    for c in range(NC):
        gpos = ((kk // P) * NC + c) * P + kk % P
        out_pos[gpos] = res.results[c]["out"]
    out_full = np.empty((N, H), np.float32)
    out_full[order] = out_pos[NDUM:]
    return out_full, res


def kernel(X, Wq, Wk, Wv, edge_index):
    out, _ = run(X, Wq, Wk, Wv, edge_index, trace=False)
    return out
